# revision 3
# baseline (speedup 1.0000x reference)
"""PointNet Feature Propagation (3-NN interpolate + shared MLP) on 8 TRN2 cores.

Strategy (data-parallel, 8 shards = 4 batches x 2 target-halves):
  - Stage 1 (PE): negative squared distances via Gram identity, computed with a
    3-level bf16 split of coordinates/norms (K=24 rows) so bf16 matmuls at full
    PE rate give ~4e-6 absolute accuracy -> top-8 candidate superset is safe.
  - Selection (DVE): max + max_index over the 2048-wide key rows -> top-8.
  - Exact refine: gather candidate coords (indirect DMA), recompute d^2 exactly
    as the fp32 reference does (diff, square, sum), pick exact top-3 via a
    bit-packed (d^2 | rank) sort on the 8 candidates.
  - Interpolate: inverse-distance weights (sqrt/recip matching reference), 3
    gathered source-feature rows, fused scalar*tensor+tensor accumulation.
  - MLP: x = [inter, feats_target]; two GEMMs in float32r (~1.5e-4) with ReLU.
    Activations kept transposed [C, T]; output written as [256, 4096] per core
    and untransposed on the host.
"""

import numpy as np
import ml_dtypes

B, NT, NS, CS, CT = 4, 8192, 2048, 256, 128
NT_CORE = NT // 2          # 4096 targets per core
N_CORES = 8
N_TILES = NT_CORE // 128   # 32
KROWS = 24

_COMPILED = {}


def _bf16_split3(x):
    """Split float64 array into 3 bf16 levels (h, m, l): x ~= h + m + l."""
    parts = []
    r = x.astype(np.float64).copy()
    for _ in range(3):
        p_b = r.astype(np.float32).astype(ml_dtypes.bfloat16)
        parts.append(p_b)
        r = r - p_b.astype(np.float64)
    return parts


def _host_prep(xyz_target, xyz_source, feats_target, feats_source, w1, w2):
    """Build per-core input maps."""
    in_maps = []
    w1s = np.ascontiguousarray(
        w1.reshape(3, 128, 256).transpose(1, 0, 2)).astype(np.float32)
    w2s = np.ascontiguousarray(
        w2.reshape(2, 128, 256).transpose(1, 0, 2)).astype(np.float32)

    for b in range(B):
        s64 = xyz_source[b].astype(np.float64)          # [NS, 3]
        ns = (s64 * s64).sum(1)                         # [NS]
        sh, sm, sl = _bf16_split3(s64.T)                # each [3, NS]
        nsh, nsm, nsl = _bf16_split3(ns)                # each [NS]
        srcK = np.zeros((KROWS, NS), dtype=ml_dtypes.bfloat16)
        srcK[0:3] = sh; srcK[3:6] = sm; srcK[6:9] = sh
        srcK[9:12] = sl; srcK[12:15] = sm; srcK[15:18] = sh
        srcK[18:21] = np.ones((3, NS), dtype=ml_dtypes.bfloat16)
        srcK[21] = nsh; srcK[22] = nsm; srcK[23] = nsl

        aug = np.zeros((NS, 4), dtype=np.float32)
        aug[:, 0:3] = xyz_source[b].astype(np.float32)
        fsrc = feats_source[b].astype(np.float32)

        for h in range(2):
            t = xyz_target[b, h * NT_CORE:(h + 1) * NT_CORE]    # [NT_CORE, 3]
            t64 = t.astype(np.float64)
            nt = (t64 * t64).sum(1)
            th, tm, tl = _bf16_split3(t64.T)                    # [3, NT_CORE]
            nth, ntm, ntl = _bf16_split3(nt)
            L = np.zeros((KROWS, NT_CORE), dtype=ml_dtypes.bfloat16)
            two = np.float32(2.0)
            L[0:3] = (th.astype(np.float32) * two).astype(ml_dtypes.bfloat16)
            L[3:6] = L[0:3]
            L[6:9] = (tm.astype(np.float32) * two).astype(ml_dtypes.bfloat16)
            L[9:12] = L[0:3]
            L[12:15] = L[6:9]
            L[15:18] = (tl.astype(np.float32) * two).astype(ml_dtypes.bfloat16)
            L[18] = (-nth.astype(np.float32)).astype(ml_dtypes.bfloat16)
            L[19] = (-ntm.astype(np.float32)).astype(ml_dtypes.bfloat16)
            L[20] = (-ntl.astype(np.float32)).astype(ml_dtypes.bfloat16)
            L[21:24] = -np.ones((3, NT_CORE), dtype=ml_dtypes.bfloat16)

            ftT = np.ascontiguousarray(
                feats_target[b, h * NT_CORE:(h + 1) * NT_CORE].T
            ).astype(np.float32)                                # [CT, NT_CORE]

            in_maps.append({
                "lhsTK": np.ascontiguousarray(L),
                "srcK": np.ascontiguousarray(srcK),
                "xt3": np.ascontiguousarray(t.astype(np.float32)),
                "aug": aug,
                "fsrc": fsrc,
                "ftT": ftT,
                "w1s": w1s,
                "w2s": w2s,
            })
    return in_maps


def _build_program():
    import concourse.bass as bass
    import concourse.bacc as bacc
    import concourse.mybir as mybir
    from concourse.tile import TileContext
    from concourse.masks import make_identity

    dt = mybir.dt
    AF = mybir.ActivationFunctionType
    OP = mybir.AluOpType

    nc = bacc.Bacc(None, target_bir_lowering=False)
    lhsTK_d = nc.declare_dram_parameter("lhsTK", [KROWS, NT_CORE], dt.bfloat16, isOutput=False)
    srcK_d = nc.declare_dram_parameter("srcK", [KROWS, NS], dt.bfloat16, isOutput=False)
    xt3_d = nc.declare_dram_parameter("xt3", [NT_CORE, 3], dt.float32, isOutput=False)
    aug_d = nc.declare_dram_parameter("aug", [NS, 4], dt.float32, isOutput=False)
    fsrc_d = nc.declare_dram_parameter("fsrc", [NS, CS], dt.float32, isOutput=False)
    ftT_d = nc.declare_dram_parameter("ftT", [CT, NT_CORE], dt.float32, isOutput=False)
    w1s_d = nc.declare_dram_parameter("w1s", [128, 3, 256], dt.float32, isOutput=False)
    w2s_d = nc.declare_dram_parameter("w2s", [128, 2, 256], dt.float32, isOutput=False)
    outT_d = nc.declare_dram_parameter("outT", [256, NT_CORE], dt.float32, isOutput=True)

    with TileContext(nc) as tc:
        with (
            tc.tile_pool(name="const", bufs=1) as cpool,
            tc.tile_pool(name="keys", bufs=2) as keyp,
            tc.tile_pool(name="small", bufs=3) as smallp,
            tc.tile_pool(name="feat", bufs=3) as featp,
            tc.tile_pool(name="mlp", bufs=2) as mlpp,
            tc.tile_pool(name="pd2", bufs=1, space="PSUM") as pd2p,
            tc.tile_pool(name="ptp", bufs=2, space="PSUM") as ptpp,
            tc.tile_pool(name="pmm", bufs=1, space="PSUM") as pmmp,
        ):
            # ---- constants ----
            lhsTK = cpool.tile([KROWS, NT_CORE], dt.bfloat16)
            srcK = cpool.tile([KROWS, NS], dt.bfloat16)
            nc.sync.dma_start(out=lhsTK[:], in_=lhsTK_d[:])
            nc.sync.dma_start(out=srcK[:], in_=srcK_d[:])

            w1f = cpool.tile([128, 3, 256], dt.float32)
            w2f = cpool.tile([128, 2, 256], dt.float32)
            nc.sync.dma_start(out=w1f[:], in_=w1s_d[:])
            nc.sync.dma_start(out=w2f[:], in_=w2s_d[:])
            w1r = cpool.tile([128, 3, 256], dt.float32r)
            w2r = cpool.tile([128, 2, 256], dt.float32r)
            nc.vector.tensor_copy(out=w1r[:], in_=w1f[:])
            nc.vector.tensor_copy(out=w2r[:], in_=w2f[:])

            ident = cpool.tile([128, 128], dt.float32)
            make_identity(nc, ident[:])

            iota_u = cpool.tile([128, 8], dt.uint32)
            nc.gpsimd.iota(out=iota_u[:], pattern=[[1, 8]], base=0, channel_multiplier=0)
            iota_f = cpool.tile([128, 8], dt.float32)
            nc.vector.tensor_copy(out=iota_f[:], in_=iota_u[:])

            for b8 in range(N_TILES // 4):
                interT4 = mlpp.tile([128, 2, 512], dt.float32r, tag="interT4")
                ftT4f = mlpp.tile([128, 512], dt.float32, tag="ftT4f")
                ftT4 = mlpp.tile([128, 512], dt.float32r, tag="ftT4")
                nc.sync.dma_start(out=ftT4f[:], in_=ftT_d[:, b8 * 512:(b8 + 1) * 512])
                nc.vector.tensor_copy(out=ftT4[:], in_=ftT4f[:])

                for it in range(4):
                    i = b8 * 4 + it
                    # ---- stage 1: sloppy -d^2 keys ----
                    pd2 = pd2p.tile([128, NS], dt.float32, tag="pd2")
                    for j in range(NS // 512):
                        nc.tensor.matmul(
                            out=pd2[:, j * 512:(j + 1) * 512],
                            lhsT=lhsTK[:, i * 128:(i + 1) * 128],
                            rhs=srcK[:, j * 512:(j + 1) * 512],
                            start=True, stop=True,
                        )
                    keys = keyp.tile([128, NS], dt.bfloat16, tag="keys")
                    nc.scalar.activation(out=keys[:], in_=pd2[:], func=AF.Copy)

                    mx8 = smallp.tile([128, 8], dt.bfloat16, tag="mx8")
                    idx8 = smallp.tile([128, 8], dt.uint32, tag="idx8")
                    nc.vector.max(out=mx8[:], in_=keys[:])
                    nc.vector.max_index(out=idx8[:], in_max=mx8[:], in_values=keys[:])

                    # ---- refine: exact d^2 of the 8 candidates ----
                    G = smallp.tile([128, 8, 4], dt.float32, tag="G")
                    for k in range(8):
                        nc.gpsimd.indirect_dma_start(
                            out=G[:, k, :],
                            out_offset=None,
                            in_=aug_d[:],
                            in_offset=bass.IndirectOffsetOnAxis(
                                ap=idx8[:, k:k + 1], axis=0),
                        )
                    xt = smallp.tile([128, 3], dt.float32, tag="xt")
                    nc.sync.dma_start(out=xt[:], in_=xt3_d[i * 128:(i + 1) * 128, :])

                    dx = smallp.tile([128, 3, 8], dt.float32, tag="dx")
                    for c in range(3):
                        nc.vector.tensor_scalar(
                            out=dx[:, c, :], in0=G[:, :, c], scalar1=xt[:, c:c + 1],
                            scalar2=None, op0=OP.subtract)
                    sq = smallp.tile([128, 3, 8], dt.float32, tag="sq")
                    for c in range(3):
                        nc.vector.tensor_tensor(
                            out=sq[:, c, :], in0=dx[:, c, :], in1=dx[:, c, :], op=OP.mult)
                    d2c = smallp.tile([128, 8], dt.float32, tag="d2c")
                    nc.vector.tensor_tensor(
                        out=d2c[:], in0=sq[:, 0, :], in1=sq[:, 1, :], op=OP.add)
                    nc.vector.tensor_tensor(
                        out=d2c[:], in0=d2c[:], in1=sq[:, 2, :], op=OP.add)

                    # pack: (d2 & ~7) | rank, negate (float), sort desc, unpack
                    pk_u = smallp.tile([128, 8], dt.uint32, tag="pk_u")
                    nc.vector.tensor_scalar(
                        out=pk_u[:], in0=d2c[:].bitcast(dt.uint32),
                        scalar1=0xFFFFFFF8, scalar2=None, op0=OP.bitwise_and)
                    nc.vector.tensor_tensor(
                        out=pk_u[:], in0=pk_u[:], in1=iota_u[:], op=OP.bitwise_or)
                    pk_f = smallp.tile([128, 8], dt.float32, tag="pk_f")
                    nc.vector.tensor_scalar(
                        out=pk_f[:], in0=pk_u[:].bitcast(dt.float32),
                        scalar1=-1.0, scalar2=None, op0=OP.mult)
                    sel8 = smallp.tile([128, 8], dt.float32, tag="sel8")
                    nc.vector.max(out=sel8[:], in_=pk_f[:])

                    top3u = smallp.tile([128, 3], dt.uint32, tag="top3u")
                    nc.vector.tensor_scalar(
                        out=top3u[:].bitcast(dt.float32), in0=sel8[:, 0:3],
                        scalar1=-1.0, scalar2=None, op0=OP.mult)
                    d2sel = smallp.tile([128, 3], dt.float32, tag="d2sel")
                    nc.vector.tensor_scalar(
                        out=d2sel[:].bitcast(dt.uint32), in0=top3u[:],
                        scalar1=0xFFFFFFF8, scalar2=None, op0=OP.bitwise_and)
                    rank3 = smallp.tile([128, 3], dt.uint32, tag="rank3")
                    nc.vector.tensor_scalar(
                        out=rank3[:], in0=top3u[:], scalar1=0x7, scalar2=None,
                        op0=OP.bitwise_and)
                    rank3f = smallp.tile([128, 3], dt.float32, tag="rank3f")
                    nc.vector.tensor_copy(out=rank3f[:], in_=rank3[:])
                    idx8f = smallp.tile([128, 8], dt.float32, tag="idx8f")
                    nc.vector.tensor_copy(out=idx8f[:], in_=idx8[:])

                    scratch8 = smallp.tile([128, 8], dt.float32, tag="scratch8")
                    idxsel_f = smallp.tile([128, 3], dt.float32, tag="idxsel_f")
                    for j in range(3):
                        nc.vector.scalar_tensor_tensor(
                            out=scratch8[:], in0=iota_f[:],
                            scalar=rank3f[:, j:j + 1], in1=idx8f[:],
                            op0=OP.is_equal, op1=OP.mult,
                            accum_out=idxsel_f[:, j:j + 1])
                    idxsel_u = smallp.tile([128, 3], dt.uint32, tag="idxsel_u")
                    nc.vector.tensor_copy(out=idxsel_u[:], in_=idxsel_f[:])

                    # ---- weights (reference: 1/max(sqrt(d2),eps), normalized) ----
                    d3 = smallp.tile([128, 3], dt.float32, tag="d3")
                    nc.scalar.activation(out=d3[:], in_=d2sel[:], func=AF.Sqrt)
                    nc.vector.tensor_scalar(
                        out=d3[:], in0=d3[:], scalar1=1e-10, scalar2=None, op0=OP.max)
                    inv3 = smallp.tile([128, 3], dt.float32, tag="inv3")
                    nc.vector.reciprocal(out=inv3[:], in_=d3[:])
                    sinv = smallp.tile([128, 1], dt.float32, tag="sinv")
                    nc.vector.tensor_reduce(
                        out=sinv[:], in_=inv3[:], axis=mybir.AxisListType.X, op=OP.add)
                    rs = smallp.tile([128, 1], dt.float32, tag="rs")
                    nc.vector.reciprocal(out=rs[:], in_=sinv[:])
                    w3 = smallp.tile([128, 3], dt.float32, tag="w3")
                    nc.vector.tensor_scalar(
                        out=w3[:], in0=inv3[:], scalar1=rs[:], scalar2=None, op0=OP.mult)
                    wsum = smallp.tile([128, 1], dt.float32, tag="wsum")
                    nc.vector.tensor_reduce(
                        out=wsum[:], in_=w3[:], axis=mybir.AxisListType.X, op=OP.add)
                    nc.vector.tensor_scalar(
                        out=wsum[:], in0=wsum[:], scalar1=1e-6, scalar2=None, op0=OP.add)
                    rws = smallp.tile([128, 1], dt.float32, tag="rws")
                    nc.vector.reciprocal(out=rws[:], in_=wsum[:])
                    f3 = smallp.tile([128, 3], dt.float32, tag="f3")
                    nc.vector.tensor_scalar(
                        out=f3[:], in0=w3[:], scalar1=rws[:], scalar2=None, op0=OP.mult)

                    # ---- gather + interpolate ----
                    fk = []
                    for k in range(3):
                        fk_t = featp.tile([128, CS], dt.float32, tag=f"fk{k}", name=f"fk{k}_{b8}_{it}")
                        fk.append(fk_t)
                    for k in range(3):
                        nc.gpsimd.indirect_dma_start(
                            out=fk[k][:],
                            out_offset=None,
                            in_=fsrc_d[:],
                            in_offset=bass.IndirectOffsetOnAxis(
                                ap=idxsel_u[:, k:k + 1], axis=0),
                        )
                    inter = featp.tile([128, CS], dt.float32, tag="inter")
                    nc.vector.tensor_scalar(
                        out=inter[:], in0=fk[0][:], scalar1=f3[:, 0:1],
                        scalar2=None, op0=OP.mult)
                    for k in (1, 2):
                        nc.vector.scalar_tensor_tensor(
                            out=inter[:], in0=fk[k][:], scalar=f3[:, k:k + 1],
                            in1=inter[:], op0=OP.mult, op1=OP.add)

                    # ---- transpose inter -> interT4 slots ----
                    for c in range(2):
                        ptp = ptpp.tile([128, 128], dt.float32, tag="ptp")
                        nc.tensor.transpose(
                            out=ptp[:], in_=inter[:, c * 128:(c + 1) * 128],
                            identity=ident[:])
                        nc.scalar.activation(
                            out=interT4[:, c, it * 128:(it + 1) * 128],
                            in_=ptp[:], func=AF.Copy)

                # ---- MLP over the 4-tile batch (N=512) ----
                h4 = mlpp.tile([128, 2, 512], dt.float32r, tag="h4")
                for dh in range(2):
                    p1 = pmmp.tile([128, 512], dt.float32, tag="p1")
                    nc.tensor.matmul(
                        out=p1[:], lhsT=w1r[:, 0, dh * 128:(dh + 1) * 128],
                        rhs=interT4[:, 0, :], start=True, stop=False)
                    nc.tensor.matmul(
                        out=p1[:], lhsT=w1r[:, 1, dh * 128:(dh + 1) * 128],
                        rhs=interT4[:, 1, :], start=False, stop=False)
                    nc.tensor.matmul(
                        out=p1[:], lhsT=w1r[:, 2, dh * 128:(dh + 1) * 128],
                        rhs=ftT4[:], start=False, stop=True)
                    nc.scalar.activation(out=h4[:, dh, :], in_=p1[:], func=AF.Relu)
                for eh in range(2):
                    p2 = pmmp.tile([128, 512], dt.float32, tag="p2")
                    nc.tensor.matmul(
                        out=p2[:], lhsT=w2r[:, 0, eh * 128:(eh + 1) * 128],
                        rhs=h4[:, 0, :], start=True, stop=False)
                    nc.tensor.matmul(
                        out=p2[:], lhsT=w2r[:, 1, eh * 128:(eh + 1) * 128],
                        rhs=h4[:, 1, :], start=False, stop=True)
                    o4 = mlpp.tile([128, 512], dt.float32, tag="o4")
                    nc.scalar.activation(out=o4[:], in_=p2[:], func=AF.Relu)
                    nc.sync.dma_start(
                        out=outT_d[eh * 128:(eh + 1) * 128, b8 * 512:(b8 + 1) * 512],
                        in_=o4[:])

    if not nc.is_finalized():
        nc.finalize()
    return nc


def _get_program():
    if "nc" not in _COMPILED:
        _COMPILED["nc"] = _build_program()
    return _COMPILED["nc"]


def kernel(xyz_target, xyz_source, feats_target, feats_source, w1, w2):
    from concourse.bass_utils import run_bass_kernel_spmd

    nc = _get_program()
    in_maps = _host_prep(xyz_target, xyz_source, feats_target, feats_source, w1, w2)
    res = run_bass_kernel_spmd(nc, in_maps, list(range(N_CORES))).results

    out = np.empty((B, NT, 256), dtype=np.float32)
    for b in range(B):
        for h in range(2):
            out[b, h * NT_CORE:(h + 1) * NT_CORE] = res[b * 2 + h]["outT"].T
    return out
